# revision 12
# baseline (speedup 1.0000x reference)
"""Trainium2 Bass kernel for 3x3 conv (stride 1, pad 1) + bias.

Problem: x (32,128,56,56) f32, weights (256,128,3,3) f32, bias (256,) f32
         -> out (32,256,56,56) f32.

Strategy: data-parallel over batch (4 images per core, 8 cores).
Per core: implicit GEMM. C_in=128 lives on the SBUF partition axis (the
matmul contraction dim). Each image is stored width+height zero-padded
(58x58 grid) in a flat per-image slot so that, for every 3x3 tap (kh,kw),
the conv becomes ONE shifted contiguous matmul over 8 output rows
(N = 8*58 = 464) accumulated in PSUM across the 9 taps. C_out=256 is
split into two 128-partition halves (the matmul M dim). Bias is added
during PSUM->SBUF eviction on the scalar engine.

Inputs are converted to bf16 on the host (fp32 matmul is 1/4 rate on
TRN2's PE; bf16 streams 1 row/cycle and accumulates in fp32 PSUM).

v2 head/tail schedule (from the v1 perfetto trace):
 - warmup rhs zeroed on the Vector engine (GpSimd memset arrived late);
   5 junk matmuls keep the PE HAM window busy until real data lands.
 - critical-path DMAs go on the Sync queue in dependency order:
   image-0 rows 0-9, then the h=0 weight half (wt is laid out h-major so
   that is one contiguous transfer), bias, then the rest of image 0.
 - images 1/2/3 stream on the Scalar/GpSimd/Vector queues, which also
   pre-warms those DGE rings for the tail.
 - output DMAs are batched two 8-row chunks per transfer; the final
   chunk's eviction is split across Scalar+Vector and its DMA across all
   four warm queues so the end-of-kernel dependency chain is minimal.
"""

import os
from contextlib import ExitStack

import ml_dtypes
import numpy as np

import concourse.bacc as bacc
import concourse.bass as bass
import concourse.mybir as mybir
import concourse.tile as tile
import concourse.bass_utils as bass_utils

N_CORES = 8
B, CIN, H, W = 32, 128, 56, 56
COUT = 256
BPC = B // N_CORES          # images per core
PW, PH = W + 1, H + 2       # grid 58 rows x 57 cols: one shared pad col
GRID = PW * PH              # 3306  (col 0 of each row is the zero pad;
                            #  col 57 === next row's col 0)
SLOT = GRID + 2             # +2 zero guard for the last row's col-57 read
                            #  (and the flat-slice bound of the 8x57 view)
RPC = 8                     # output rows per PSUM chunk
NCHUNK = H // RPC           # 7
NFREE = RPC * W             # 448 moving-dim elements per matmul (2D AP)
KK = 9                      # 3x3 taps
WH = KK * 128               # one h-half of the weights (1152 cols)

DT = mybir.dt.bfloat16
NPDT = ml_dtypes.bfloat16

# input split points (elements within a SLOT) chosen so each chunk's
# rows are covered: chunk c needs rows RPC*c .. RPC*c+9
P0A = 10 * PW + 2           # rows 0-9 (chunk 0)
P0B = 18 * PW + 2           # rows -17 (chunk 1)
P01 = 34 * PW + 2           # rows -33 (chunks 2-3)

_CACHE: dict = {}


def _build():
    """Build the per-core Bass program (same program on all 8 cores)."""
    nc = bacc.Bacc("TRN2", target_bir_lowering=False, debug=False,
                   num_devices=N_CORES)
    f32 = mybir.dt.float32
    xp = nc.dram_tensor("xp", [BPC, CIN, SLOT], DT, kind="ExternalInput").ap()
    wt = nc.dram_tensor("wt", [CIN, 2 * WH], DT, kind="ExternalInput").ap()
    b2 = nc.dram_tensor("b2", [128, 2], f32, kind="ExternalInput").ap()
    out = nc.dram_tensor("out", [BPC, COUT, H, W], f32,
                         kind="ExternalOutput").ap()

    with tile.TileContext(nc) as tc, ExitStack() as ctx:
        const_pool = ctx.enter_context(tc.tile_pool(name="const", bufs=1))
        xpool = ctx.enter_context(tc.tile_pool(name="xp_pool", bufs=1))
        epool = ctx.enter_context(tc.tile_pool(name="epool", bufs=4))
        psum = ctx.enter_context(
            tc.tile_pool(name="psum", bufs=7, space="PSUM"))
        wupool = ctx.enter_context(
            tc.tile_pool(name="wupool", bufs=1, space="PSUM"))

        wbuf = const_pool.tile([CIN, 2 * WH], DT)
        xbuf = xpool.tile([CIN, BPC * SLOT], DT)
        bbuf = const_pool.tile([128, 2], f32)

        # HAM warmup: junk matmuls from ~t0 so the PE clock-gate is at 8/8
        # (2.4 GHz) when the real stream starts. Vector memset is ready
        # earliest (GpSimd's was ~0.5us late in the v1 trace).
        wrm = const_pool.tile([128, 512], DT)
        nc.vector.memset(wrm[:], 0)
        wps = wupool.tile([128, 512], f32)
        for _ in range(4):
            nc.tensor.matmul(wps[:], wrm[:, :128], wrm[:],
                             start=True, stop=True)

        # The 16 physical DMA engines are one shared pool that round-robins
        # over ALL outstanding transfers (any queue), so every early issue
        # steals bandwidth from the first-chunk data. Only the critical
        # transfers are issued up front (Sync queue, consumption order);
        # the bulk is issued on the Scalar engine AFTER the first eviction
        # (program order on the engine = a free data-dependent stall).
        nc.sync.dma_start(xbuf[:, :P0A], xp[0][:, :P0A])
        nc.sync.dma_start(wbuf[:, :WH], wt[:, :WH])
        nc.sync.dma_start(xbuf[:, P0A:P0B], xp[0][:, P0A:P0B])
        nc.sync.dma_start(bbuf[:], b2)
        nc.sync.dma_start(xbuf[:, P0B:P01], xp[0][:, P0B:P01])
        nc.sync.dma_start(xbuf[:, P01:SLOT], xp[0][:, P01:SLOT])

        pss = [psum.tile([128, NFREE], f32, name=f"ps{i}", tag=f"ps{i}",
                         bufs=1)
               for i in range(NCHUNK)]
        evs = [epool.tile([128, 2 * NFREE], f32, name=f"ev{i}", tag=f"ev{i}",
                          bufs=1)
               for i in range(4)]
        ipair = 0
        for n in range(BPC):
            for h in range(2):
                for c in range(NCHUNK):
                    ps = pss[c]
                    for k in range(KK):
                        kh, kw = divmod(k, 3)
                        s = n * SLOT + PW * (RPC * c + kh) + kw
                        rhs = xbuf[:, s:s + RPC * PW].rearrange(
                            "p (r c) -> p r c", c=PW)[:, :, :W]
                        nc.tensor.matmul(
                            ps[:],
                            wbuf[:, h * WH + k * 128:h * WH + k * 128 + 128],
                            rhs,
                            start=(k == 0),
                            stop=(k == KK - 1),
                        )
                    final_nh = (n == BPC - 1) and (h == 1)
                    last = final_nh and (c == NCHUNK - 1)
                    if not last:
                        ev = evs[ipair % 4]
                        side = c % 2
                        nc.scalar.activation(
                            ev[:, side * NFREE:(side + 1) * NFREE], ps[:],
                            mybir.ActivationFunctionType.Identity,
                            bias=bbuf[:, h:h + 1])
                        if n == 0 and h == 0 and c == 0:
                            # bulk input, issued here so the transfers only
                            # hit the DMA pool after the critical window
                            nc.scalar.dma_start(wbuf[:, WH:], wt[:, WH:])
                            for nn in range(1, BPC):
                                nc.scalar.dma_start(
                                    xbuf[:, nn * SLOT:(nn + 1) * SLOT],
                                    xp[nn])
                        if side == 1 or c == NCHUNK - 1:
                            c0 = c - side
                            od = out[n, h * 128:(h + 1) * 128,
                                     c0 * RPC:(c + 1) * RPC].rearrange(
                                         "c r w -> c (r w)")
                            # the final (n,h)'s pairs go on the Sync queue
                            # so the Scalar queue is drained at the tail
                            eng = nc.sync if (final_nh and c >= 2) else \
                                nc.scalar
                            eng.dma_start(od, ev[:, :(side + 1) * NFREE])
                            ipair += 1
                    else:
                        # final chunk: eviction split across two engines,
                        # halves DMA'd on the two drained queues so the
                        # tail dependency chain is short.
                        ev = evs[ipair % 4]
                        half = NFREE // 2
                        nc.scalar.activation(
                            ev[:, :half], ps[:, :half],
                            mybir.ActivationFunctionType.Identity,
                            bias=bbuf[:, h:h + 1])
                        nc.vector.tensor_scalar_add(
                            ev[:, half:NFREE], ps[:, half:], bbuf[:, h:h + 1])
                        od = out[n, h * 128:(h + 1) * 128,
                                 c * RPC:(c + 1) * RPC].rearrange(
                                     "c r w -> c (r w)")
                        nc.sync.dma_start(od[:, :half], ev[:, :half])
                        nc.scalar.dma_start(od[:, half:], ev[:, half:NFREE])
    nc.compile()
    return nc


def _prep(x, weights, bias):
    """Host-side reshape/pad/cast into the device layouts."""
    xpad = np.zeros((B, CIN, SLOT), dtype=NPDT)
    grid = xpad[:, :, :GRID].reshape(B, CIN, PH, PW)
    # rows 1..56 hold the image; col 0 is the zero pad column (col 57 of a
    # row aliases the next row's col 0, so one pad column serves both edges)
    grid[:, :, 1:1 + H, 1:1 + W] = np.asarray(x).astype(NPDT)
    # weights (co, ci, kh, kw) -> (ci, h, kh*kw, m) flat, h-major so one
    # contiguous DMA covers all nine taps of an output-channel half
    wv = np.asarray(weights).reshape(2, 128, CIN, 3, 3)
    wt = np.ascontiguousarray(wv.transpose(2, 0, 3, 4, 1)).reshape(
        CIN, 2 * WH).astype(NPDT)
    b2 = np.ascontiguousarray(
        np.asarray(bias).astype(np.float32).reshape(2, 128).T)
    return xpad, wt, b2


def kernel(x, weights, bias):
    if "nc" not in _CACHE:
        _CACHE["nc"] = _build()
    nc = _CACHE["nc"]
    xpad, wt, b2 = _prep(x, weights, bias)
    in_maps = [
        {"xp": xpad[i * BPC:(i + 1) * BPC], "wt": wt, "b2": b2}
        for i in range(N_CORES)
    ]
    res = bass_utils.run_bass_kernel_spmd(
        nc, in_maps, core_ids=list(range(N_CORES)),
        trace=bool(int(os.environ.get("CONV_TRACE", "0"))),
    )
    if os.environ.get("CONV_TRACE"):
        _CACHE["last_result"] = res
    return np.concatenate([r["out"] for r in res.results], axis=0)


# revision 15
# speedup vs baseline: 1.2526x; 1.2526x over previous
"""Trainium2 Bass kernel for 3x3 conv (stride 1, pad 1) + bias.

Problem: x (32,128,56,56) f32, weights (256,128,3,3) f32, bias (256,) f32
         -> out (32,256,56,56) f32.

Strategy: data-parallel over batch (4 images per core, 8 cores).
Per core: implicit GEMM. C_in=128 lives on the SBUF partition axis (the
matmul contraction dim). Each image is stored width+height zero-padded
(58x58 grid) in a flat per-image slot so that, for every 3x3 tap (kh,kw),
the conv becomes ONE shifted contiguous matmul over 8 output rows
(N = 8*58 = 464) accumulated in PSUM across the 9 taps. C_out=256 is
split into two 128-partition halves (the matmul M dim). Bias is added
during PSUM->SBUF eviction on the scalar engine.

Inputs are converted to bf16 on the host (fp32 matmul is 1/4 rate on
TRN2's PE; bf16 streams 1 row/cycle and accumulates in fp32 PSUM).

v2 head/tail schedule (from the v1 perfetto trace):
 - warmup rhs zeroed on the Vector engine (GpSimd memset arrived late);
   5 junk matmuls keep the PE HAM window busy until real data lands.
 - critical-path DMAs go on the Sync queue in dependency order:
   image-0 rows 0-9, then the h=0 weight half (wt is laid out h-major so
   that is one contiguous transfer), bias, then the rest of image 0.
 - images 1/2/3 stream on the Scalar/GpSimd/Vector queues, which also
   pre-warms those DGE rings for the tail.
 - output DMAs are batched two 8-row chunks per transfer; the final
   chunk's eviction is split across Scalar+Vector and its DMA across all
   four warm queues so the end-of-kernel dependency chain is minimal.
"""

import os
from contextlib import ExitStack

import ml_dtypes
import numpy as np

import concourse.bacc as bacc
import concourse.bass as bass
import concourse.mybir as mybir
import concourse.tile as tile
import concourse.bass_utils as bass_utils

N_CORES = 8
B, CIN, H, W = 32, 128, 56, 56
COUT = 256
BPC = B // N_CORES          # images per core
PW, PH = W + 1, H + 2       # grid 58 rows x 57 cols: one shared pad col
GRID = PW * PH              # 3306  (col 0 of each row is the zero pad;
                            #  col 57 === next row's col 0)
SLOT = GRID + 2             # +2 zero guard for the last row's col-57 read
                            #  (and the flat-slice bound of the 8x57 view)
RPC = 8                     # output rows per PSUM chunk
NCHUNK = H // RPC           # 7
NFREE = RPC * W             # 448 moving-dim elements per matmul (2D AP)
KK = 9                      # 3x3 taps
WH = KK * 128               # one h-half of the weights (1152 cols)

DT = mybir.dt.bfloat16
NPDT = ml_dtypes.bfloat16

# input split points (elements within a SLOT) chosen so each chunk's
# rows are covered: chunk c needs rows RPC*c .. RPC*c+9
P0A = 10 * PW + 2           # rows 0-9 (chunk 0)
P0B = 18 * PW + 2           # rows -17 (chunk 1)
P01 = 34 * PW + 2           # rows -33 (chunks 2-3)

_CACHE: dict = {}


def _build():
    """Build the per-core Bass program (same program on all 8 cores)."""
    nc = bacc.Bacc("TRN2", target_bir_lowering=False, debug=False,
                   num_devices=N_CORES)
    f32 = mybir.dt.float32
    xp = nc.dram_tensor("xp", [BPC, CIN, SLOT], DT, kind="ExternalInput").ap()
    wt = nc.dram_tensor("wt", [CIN, 2 * WH], DT, kind="ExternalInput").ap()
    b2 = nc.dram_tensor("b2", [128, 2], f32, kind="ExternalInput").ap()
    out = nc.dram_tensor("out", [BPC, COUT, H, W], f32,
                         kind="ExternalOutput").ap()

    with tile.TileContext(nc) as tc, ExitStack() as ctx:
        const_pool = ctx.enter_context(tc.tile_pool(name="const", bufs=1))
        xpool = ctx.enter_context(tc.tile_pool(name="xp_pool", bufs=1))
        epool = ctx.enter_context(tc.tile_pool(name="epool", bufs=4))
        psum = ctx.enter_context(
            tc.tile_pool(name="psum", bufs=7, space="PSUM"))
        wupool = ctx.enter_context(
            tc.tile_pool(name="wupool", bufs=1, space="PSUM"))

        wbuf = const_pool.tile([CIN, 2 * WH], DT)
        xbuf = xpool.tile([CIN, BPC * SLOT], DT)
        bbuf = const_pool.tile([128, 2], f32)

        # HAM warmup: junk matmuls from ~t0 so the PE clock-gate is at 8/8
        # (2.4 GHz) when the real stream starts. Vector memset is ready
        # earliest (GpSimd's was ~0.5us late in the v1 trace).
        wrm = const_pool.tile([128, 512], DT)
        nc.vector.memset(wrm[:], 0)
        wps = wupool.tile([128, 512], f32)
        for _ in range(7):
            nc.tensor.matmul(wps[:], wrm[:, :128], wrm[:],
                             start=True, stop=True)

        # The 16 physical DMA engines are one shared pool that round-robins
        # over ALL outstanding transfers (any queue), so every early issue
        # steals bandwidth from the first-chunk data. Only the critical
        # transfers are issued up front (Sync queue, consumption order);
        # the bulk is issued on the Scalar engine AFTER the first eviction
        # (program order on the engine = a free data-dependent stall).
        nc.sync.dma_start(xbuf[:, :P0A], xp[0][:, :P0A])
        nc.sync.dma_start(wbuf[:, :WH], wt[:, :WH])
        nc.sync.dma_start(xbuf[:, P0A:P0B], xp[0][:, P0A:P0B])
        nc.sync.dma_start(bbuf[:], b2)
        nc.sync.dma_start(xbuf[:, P0B:P01], xp[0][:, P0B:P01])
        nc.sync.dma_start(xbuf[:, P01:SLOT], xp[0][:, P01:SLOT])
        nc.sync.dma_start(wbuf[:, WH:], wt[:, WH:])
        for nn in range(1, BPC):
            nc.sync.dma_start(xbuf[:, nn * SLOT:(nn + 1) * SLOT], xp[nn])

        pss = [psum.tile([128, NFREE], f32, name=f"ps{i}", tag=f"ps{i}",
                         bufs=1)
               for i in range(NCHUNK)]
        evs = [epool.tile([128, 2 * NFREE], f32, name=f"ev{i}", tag=f"ev{i}",
                          bufs=1)
               for i in range(4)]
        ipair = 0
        for n in range(BPC):
            for h in range(2):
                for c in range(NCHUNK):
                    ps = pss[c]
                    for k in range(KK):
                        kh, kw = divmod(k, 3)
                        s = n * SLOT + PW * (RPC * c + kh) + kw
                        rhs = xbuf[:, s:s + RPC * PW].rearrange(
                            "p (r c) -> p r c", c=PW)[:, :, :W]
                        nc.tensor.matmul(
                            ps[:],
                            wbuf[:, h * WH + k * 128:h * WH + k * 128 + 128],
                            rhs,
                            start=(k == 0),
                            stop=(k == KK - 1),
                        )
                    final_nh = (n == BPC - 1) and (h == 1)
                    last = final_nh and (c == NCHUNK - 1)
                    if not last:
                        ev = evs[ipair % 4]
                        side = c % 2
                        nc.scalar.activation(
                            ev[:, side * NFREE:(side + 1) * NFREE], ps[:],
                            mybir.ActivationFunctionType.Identity,
                            bias=bbuf[:, h:h + 1])
                        if side == 1 or c == NCHUNK - 1:
                            c0 = c - side
                            od = out[n, h * 128:(h + 1) * 128,
                                     c0 * RPC:(c + 1) * RPC].rearrange(
                                         "c r w -> c (r w)")
                            # the final (n,h)'s pairs go on the Sync queue
                            # so the Scalar queue is drained at the tail
                            eng = nc.sync if (final_nh and c >= 2) else \
                                nc.scalar
                            eng.dma_start(od, ev[:, :(side + 1) * NFREE])
                            ipair += 1
                    else:
                        # final chunk: eviction split across two engines,
                        # halves DMA'd on the two drained queues so the
                        # tail dependency chain is short.
                        ev = evs[ipair % 4]
                        half = NFREE // 2
                        nc.scalar.activation(
                            ev[:, :half], ps[:, :half],
                            mybir.ActivationFunctionType.Identity,
                            bias=bbuf[:, h:h + 1])
                        nc.vector.tensor_scalar_add(
                            ev[:, half:NFREE], ps[:, half:], bbuf[:, h:h + 1])
                        od = out[n, h * 128:(h + 1) * 128,
                                 c * RPC:(c + 1) * RPC].rearrange(
                                     "c r w -> c (r w)")
                        nc.sync.dma_start(od[:, :half], ev[:, :half])
                        nc.scalar.dma_start(od[:, half:], ev[:, half:NFREE])
    nc.compile()
    return nc


def _prep(x, weights, bias):
    """Host-side reshape/pad/cast into the device layouts."""
    xpad = np.zeros((B, CIN, SLOT), dtype=NPDT)
    grid = xpad[:, :, :GRID].reshape(B, CIN, PH, PW)
    # rows 1..56 hold the image; col 0 is the zero pad column (col 57 of a
    # row aliases the next row's col 0, so one pad column serves both edges)
    grid[:, :, 1:1 + H, 1:1 + W] = np.asarray(x).astype(NPDT)
    # weights (co, ci, kh, kw) -> (ci, h, kh*kw, m) flat, h-major so one
    # contiguous DMA covers all nine taps of an output-channel half
    wv = np.asarray(weights).reshape(2, 128, CIN, 3, 3)
    wt = np.ascontiguousarray(wv.transpose(2, 0, 3, 4, 1)).reshape(
        CIN, 2 * WH).astype(NPDT)
    b2 = np.ascontiguousarray(
        np.asarray(bias).astype(np.float32).reshape(2, 128).T)
    return xpad, wt, b2


def kernel(x, weights, bias):
    if "nc" not in _CACHE:
        _CACHE["nc"] = _build()
    nc = _CACHE["nc"]
    xpad, wt, b2 = _prep(x, weights, bias)
    in_maps = [
        {"xp": xpad[i * BPC:(i + 1) * BPC], "wt": wt, "b2": b2}
        for i in range(N_CORES)
    ]
    res = bass_utils.run_bass_kernel_spmd(
        nc, in_maps, core_ids=list(range(N_CORES)),
        trace=bool(int(os.environ.get("CONV_TRACE", "0"))),
    )
    if os.environ.get("CONV_TRACE"):
        _CACHE["last_result"] = res
    return np.concatenate([r["out"] for r in res.results], axis=0)
